# revision 1
# baseline (speedup 1.0000x reference)
"""LSTM encoder kernel for TRN2 (8 NeuronCores, data-parallel over batch).

Reference computation:
  x = feats @ W_embed.T + b_embed            [B,T,F] -> [B,T,E]
  per t: gates = x_t @ W_ih.T + b_ih + h @ W_hh.T + b_hh
         i,f,g,o = split(gates); c = sig(f)*c + sig(i)*tanh(g); h = sig(o)*tanh(c)
  out = stacked h                            [B,T,H]

B=256, T=64, F=2048, E=512, H=512.  Shard B over 8 cores (B_L=32 each).

Device-side design (per core):
  Phase 1: xT[E, T*B_L] = W_emb @ featsT in float32r (feats pre-transposed on
           host so the per-step slice xT[:, t*32:(t+1)*32] is directly the
           lhsT of step t).
  Phase 2: per step, col-tiled fp16 matmuls (tile_position=(0,32j)) produce
           gates in PSUM with layout [(j=h-chunk, b) partitions, (gate,h')
           free] so all elementwise LSTM math runs on full 128 partitions.
           Gate columns are pre-permuted on host to (i,f,o,g) so one sigmoid
           covers free[0:384] and one tanh covers free[384:512].
           h is re-transposed for the next step with 4 row-tiled PE
           transposes.
  Matmuls: embedding fp32r (1 cyc/row at N>=256), recurrence fp16 (1 cyc/row,
  col-tiling legal, ~1.4e-3 rel err end to end).  Elementwise fp32.
"""

import numpy as np

B, T, F, E, H = 256, 64, 2048, 512, 512
NC = 8
BL = B // NC          # 32 batch rows per core
G4 = 4 * H            # 2048 gate columns
NJ = 4                # h-chunks of 128 (and batch col-groups of 32)
HP = H // NJ          # 128

_prog_cache = {}


def _build_program(has_bias: bool, bench_loop: int = 0, mode: str = "all"):
    import concourse.bass as bass
    import concourse.tile as tile
    from concourse import bacc, mybir
    from concourse.masks import make_identity
    from contextlib import ExitStack

    f32 = mybir.dt.float32
    f32r = mybir.dt.float32r
    f16 = mybir.dt.float16

    nc = bacc.Bacc("TRN2", target_bir_lowering=False, debug=False)

    featsT = nc.dram_tensor("featsT", [F, T * BL], f16, kind="ExternalInput").ap()
    wembT = nc.dram_tensor("wembT", [F, E], f16, kind="ExternalInput").ap()
    wrec = nc.dram_tensor("wrec", [2 * E, G4], f16, kind="ExternalInput").ap()
    if has_bias:
        biasg = nc.dram_tensor("biasg", [128, 512], f32, kind="ExternalInput").ap()
    out = nc.dram_tensor("out", [T, 128, HP], f32, kind="ExternalOutput").ap()

    with tile.TileContext(nc) as tc:
        with ExitStack() as ctx:
            const_pool = ctx.enter_context(tc.tile_pool(name="const", bufs=1))
            state_pool = ctx.enter_context(tc.tile_pool(name="state", bufs=1))
            embin_pool = ctx.enter_context(tc.tile_pool(name="emb_in", bufs=2))
            pg_pool = ctx.enter_context(
                tc.tile_pool(name="pg", bufs=4, space="PSUM"))
            embps_pool = pg_pool
            pt_pool = ctx.enter_context(
                tc.tile_pool(name="pt", bufs=4, space="PSUM"))
            ew_pool = ctx.enter_context(tc.tile_pool(name="ew", bufs=3))

            # I32 replicated per 32-partition block (transpose rhs must share
            # the lhsT base partition).
            ident = const_pool.tile([128, 32], f32)
            for j in range(NJ):
                make_identity(nc, ident[32 * j:32 * (j + 1), :])

            xT = const_pool.tile([128, NJ, T * BL], f16)         # 2 MB
            wrec_sb = const_pool.tile([128, 8, G4], f16)         # 4 MB
            nc.sync.dma_start(
                wrec_sb[:], wrec.rearrange("(ko p) n -> p ko n", p=128)
            )
            wemb_sb = const_pool.tile([128, 16, E], f16)         # 2 MB
            nc.sync.dma_start(
                wemb_sb[:], wembT.rearrange("(ko p) m -> p ko m", p=128)
            )
            if has_bias:
                bias_sb = const_pool.tile([128, 512], f32)
                nc.sync.dma_start(bias_sb[:], biasg[:])

            # h transposed: hT[h', j*32+b]; lhsT of K-subtile ko is
            # hT[:, 32*ko:32*(ko+1)]
            hT = state_pool.tile([128, NJ * BL], f16)
            c_sb = state_pool.tile([128, HP], f32)               # cell state

            def body():
                nc.vector.memzero(hT[:])
                nc.vector.memzero(c_sb[:])

                if mode == "rec":
                    pass  # skip embedding (xT left stale)
                # ---- phase 1: xT = W_emb @ featsT (fp32r) ----
                NCH = 256
                NCHUNKS = 0 if mode == "rec" else (T * BL) // NCH
                featsT_r = featsT.rearrange("(ko p) n -> p ko n", p=128)
                for nch in range(NCHUNKS):
                    rhs = embin_pool.tile([128, 16, NCH], f16, tag="embrhs")
                    nc.sync.dma_start(
                        rhs[:], featsT_r[:, :, nch * NCH:(nch + 1) * NCH]
                    )
                    for m in range(NJ):
                        ps = embps_pool.tile([128, 512], f32, tag="psg", name=f"embps_{nch}_{m}")[:, :NCH]
                        for ko in range(16):
                            nc.tensor.matmul(
                                ps[:],
                                wemb_sb[:, ko, m * 128:(m + 1) * 128],
                                rhs[:, ko, :],
                                start=(ko == 0),
                                stop=(ko == 15),
                            )
                        nc.vector.tensor_copy(
                            xT[:, m, nch * NCH:(nch + 1) * NCH], ps[:]
                        )

                # ---- phase 2: recurrence (fp16 col-tiled matmuls) ----
                # x-part MMs are emitted XAHEAD steps early so the PE has
                # independent work during step t's elementwise chain.
                XAHEAD = 3
                NT = 0 if mode == "emb" else T
                psgs = {}

                def emit_x(t):
                    psgs[t] = pg_pool.tile([128, 512], f32, tag="psg", name=f"psg_{t}")
                    for ko in range(4):
                        lhsT = xT[:, ko, t * BL:(t + 1) * BL]
                        for j in range(NJ):
                            nc.tensor.matmul(
                                psgs[t][32 * j:32 * (j + 1), :],
                                lhsT,
                                wrec_sb[:, ko + 4, j * 512:(j + 1) * 512],
                                start=(ko == 0), stop=False,
                                tile_position=(0, 32 * j),
                                skip_group_check=True,
                            )

                for t in range(min(XAHEAD, NT)):
                    emit_x(t)
                for t in range(NT):
                    psg = psgs.pop(t)
                    for ko in range(4):
                        lhsT = hT[:, 32 * ko:32 * (ko + 1)]
                        for j in range(NJ):
                            nc.tensor.matmul(
                                psg[32 * j:32 * (j + 1), :],
                                lhsT,
                                wrec_sb[:, ko, j * 512:(j + 1) * 512],
                                start=False, stop=(ko == 3),
                                tile_position=(0, 32 * j),
                                skip_group_check=True,
                            )
                    if t + XAHEAD < NT:
                        emit_x(t + XAHEAD)
                    if has_bias:
                        nc.vector.tensor_add(psg[:], psg[:], bias_sb[:])
                    if mode == "noew":
                        h_new = ew_pool.tile([128, HP], f32, tag="h_new")
                        nc.vector.tensor_copy(h_new[:], psg[:, 0:HP])
                    else:
                        # gate order (f,i,g,o): f [0:128], i [128:256],
                        # g [256:384], o [384:512]
                        acts = ew_pool.tile([128, 512], f32, tag="acts")
                        nc.scalar.activation(
                            acts[:, 0:256], psg[:, 0:256],
                            mybir.ActivationFunctionType.Sigmoid,
                        )
                        nc.scalar.activation(
                            acts[:, 256:384], psg[:, 256:384],
                            mybir.ActivationFunctionType.Tanh,
                        )
                        nc.scalar.activation(
                            acts[:, 384:512], psg[:, 384:512],
                            mybir.ActivationFunctionType.Sigmoid,
                        )
                        fc = ew_pool.tile([128, HP], f32, tag="fc")
                        nc.vector.tensor_mul(fc[:], acts[:, 0:128], c_sb[:])
                        ig = ew_pool.tile([128, HP], f32, tag="ig")
                        nc.vector.tensor_mul(ig[:], acts[:, 128:256],
                                             acts[:, 256:384])
                        nc.vector.tensor_add(c_sb[:], fc[:], ig[:])
                        tanh_c = ew_pool.tile([128, HP], f32, tag="tanh_c")
                        nc.scalar.activation(
                            tanh_c[:], c_sb[:],
                            mybir.ActivationFunctionType.Tanh,
                        )
                        h_new = ew_pool.tile([128, HP], f32, tag="h_new")
                        nc.vector.tensor_mul(h_new[:], acts[:, 384:512],
                                             tanh_c[:])
                    nc.sync.dma_start(out[t], h_new[:])
                    if mode != "notr" and t + 1 < NT:
                        for j in range(NJ):
                            tp = pt_pool.tile([128, BL], f32, tag="tp")
                            nc.tensor.matmul(
                                tp[:],
                                h_new[32 * j:32 * (j + 1), :],
                                ident[32 * j:32 * (j + 1), :],
                                is_transpose=True,
                                tile_position=(32 * j, 0),
                            )
                            nc.vector.tensor_copy(
                                hT[:, 32 * j:32 * (j + 1)], tp[:])

            if bench_loop:
                with tc.For_i(0, bench_loop, 1):
                    body()
            else:
                body()

    nc.compile()
    return nc


def _prep_inputs(feats_videos, W_embed, b_embed, W_ih, W_hh, b_ih, b_hh):
    """Host-side shard + relayout. Returns (in_maps, has_bias)."""
    f32 = np.float32
    # Combined recurrence weights: rows 0:511 = W_hh.T (h part),
    # rows 512:1023 = W_ih.T (x part).  Columns reordered to
    # col = jchunk*512 + gatepos*128 + h', gate order (i,f,o,g).
    W_cat = np.concatenate([W_hh.T, W_ih.T], axis=0).astype(f32)  # [1024, 2048]
    arr = W_cat.reshape(2 * E, 4, NJ, HP)       # [k, gate_orig, jchunk, h']
    arr = arr[:, [1, 0, 2, 3], :, :]            # gate order -> (f, i, g, o)
    wrec_np = np.ascontiguousarray(
        arr.transpose(0, 2, 1, 3).reshape(2 * E, G4)
    ).astype(np.float16)

    wembT_np = np.ascontiguousarray(W_embed.T).astype(np.float16)  # [F, E]

    # total gate bias, in the same [(j,b), (gatepos,h')] layout as psum
    b_g = (W_ih @ b_embed + b_ih + b_hh).astype(f32)        # [2048]
    has_bias = bool(np.any(b_g))
    bias_np = None
    if has_bias:
        bg = b_g.reshape(4, NJ, HP)[[1, 0, 2, 3], :, :]     # [gatepos, j, h']
        bias_tile = np.empty((128, 512), dtype=f32)
        for j in range(NJ):
            for gp in range(4):
                bias_tile[32 * j:32 * (j + 1), 128 * gp:128 * (gp + 1)] = bg[gp, j][None, :]
        bias_np = bias_tile

    in_maps = []
    for c in range(NC):
        fl = feats_videos[c * BL:(c + 1) * BL]              # [32, 64, 2048]
        featsT_np = np.ascontiguousarray(
            fl.transpose(2, 1, 0).reshape(F, T * BL)
        ).astype(np.float16)
        m = {"featsT": featsT_np, "wembT": wembT_np, "wrec": wrec_np}
        if has_bias:
            m["biasg"] = bias_np
        in_maps.append(m)
    return in_maps, has_bias


def kernel(feats_videos, W_embed, b_embed, W_ih, W_hh, b_ih, b_hh):
    import sys
    if "/opt/trn_rl_repo" not in sys.path:
        sys.path.insert(0, "/opt/trn_rl_repo")
    from concourse.bass_utils import run_bass_kernel_spmd

    feats_videos = np.asarray(feats_videos, dtype=np.float32)
    W_embed = np.asarray(W_embed, dtype=np.float32)
    b_embed = np.asarray(b_embed, dtype=np.float32)
    W_ih = np.asarray(W_ih, dtype=np.float32)
    W_hh = np.asarray(W_hh, dtype=np.float32)
    b_ih = np.asarray(b_ih, dtype=np.float32)
    b_hh = np.asarray(b_hh, dtype=np.float32)

    in_maps, has_bias = _prep_inputs(
        feats_videos, W_embed, b_embed, W_ih, W_hh, b_ih, b_hh
    )
    if has_bias not in _prog_cache:
        _prog_cache[has_bias] = _build_program(has_bias)
    nc = _prog_cache[has_bias]

    res = run_bass_kernel_spmd(nc, in_maps, list(range(NC)))
    outs = []
    for c in range(NC):
        r = res.results[c]["out"]                            # [64, 128, 128]
        o = r.reshape(T, NJ, BL, HP).transpose(2, 0, 1, 3).reshape(BL, T, H)
        outs.append(o)
    return np.concatenate(outs, axis=0).astype(np.float32)   # [256, 64, 512]



# revision 26
# speedup vs baseline: 68.4656x; 68.4656x over previous
"""LSTM encoder kernel for TRN2 (8 NeuronCores, data-parallel over batch).

Reference computation:
  x = feats @ W_embed.T + b_embed            [B,T,F] -> [B,T,E]
  per t: gates = x_t @ W_ih.T + b_ih + h @ W_hh.T + b_hh
         i,f,g,o = split(gates); c = sig(f)*c + sig(i)*tanh(g); h = sig(o)*tanh(c)
  out = stacked h                            [B,T,H]

B=256, T=64, F=2048, E=512, H=512.  Shard B over 8 cores (B_L=32 each).

Device-side design (per core):
  Phase 1: xT[E, T*B_L] = W_emb @ featsT in fp16 (feats pre-transposed on
           host so the per-step slice xT[:, t*32:(t+1)*32] is directly the
           lhsT of step t).
  Phase 2: per step, col-tiled fp16 matmuls (tile_position=(0,32j)) produce
           gates in PSUM with layout [(j=h-chunk, b) partitions, (gate,h')
           free] so all elementwise LSTM math runs on full 128 partitions.
           Gate columns are pre-permuted on host to (f,i,g,o) so one sigmoid
           covers free[0:256] (f,i), one tanh free[256:384] (g), one sigmoid
           free[384:512] (o).
  h-transposition: instead of transposing h_new, transpose s_o and tanh_c
           (each one full 128x128 fp16 PE transpose) and multiply them in
           the TRANSPOSED layout: hT = s_oT * tanh_cT.  This writes the f16
           recurrence state directly (no PSUM->SBUF copy of h) and the
           output DMA sends the same f16 tile.
"""

import numpy as np

B, T, F, E, H = 256, 64, 2048, 512, 512
NC = 8
BL = B // NC          # 32 batch rows per core
G4 = 4 * H            # 2048 gate columns
NJ = 4                # h-chunks of 128 (and batch col-groups of 32)
HP = H // NJ          # 128

_prog_cache = {}


def _build_program(has_bias: bool, bench_loop: int = 0, mode: str = "all"):
    import concourse.bass as bass
    import concourse.tile as tile
    from concourse import bacc, mybir
    from concourse.masks import make_identity
    from contextlib import ExitStack

    f32 = mybir.dt.float32
    f16 = mybir.dt.float16

    nc = bacc.Bacc("TRN2", target_bir_lowering=False, debug=False)

    featsT = nc.dram_tensor("featsT", [F, T * BL], f16, kind="ExternalInput").ap()
    wembT = nc.dram_tensor("wembT", [F, E], f16, kind="ExternalInput").ap()
    wrec = nc.dram_tensor("wrec", [2 * E, G4], f16, kind="ExternalInput").ap()
    if has_bias:
        biasg = nc.dram_tensor("biasg", [128, 512], f32, kind="ExternalInput").ap()
    out = nc.dram_tensor("out", [T, 128, HP], f16, kind="ExternalOutput").ap()

    with tile.TileContext(nc) as tc:
        with ExitStack() as ctx:
            const_pool = ctx.enter_context(tc.tile_pool(name="const", bufs=1))
            state_pool = ctx.enter_context(tc.tile_pool(name="state", bufs=2))
            embin_pool = ctx.enter_context(tc.tile_pool(name="emb_in", bufs=3))
            pgfi_pool = ctx.enter_context(
                tc.tile_pool(name="pgfi", bufs=2, space="PSUM"))
            pggo_pool = ctx.enter_context(
                tc.tile_pool(name="pggo", bufs=2, space="PSUM"))
            embps_pool = ctx.enter_context(
                tc.tile_pool(name="embp", bufs=2, space="PSUM"))
            pt_pool = ctx.enter_context(
                tc.tile_pool(name="pt", bufs=1, space="PSUM"))
            ew_pool = ctx.enter_context(tc.tile_pool(name="ew", bufs=3))

            ident16 = const_pool.tile([128, 128], f16)
            make_identity(nc, ident16[:])

            xT = const_pool.tile([128, NJ, T * BL], f16)         # 2 MB
            # DMA transfers are HBM-bandwidth serialized; what matters is
            # SERVICE ORDER = arrival order.  wemb + feats chunk 0 go first
            # (they gate the first PE work); wrec halves ride the slower
            # gpsimd SWDGE queue so they arrive after feats0 but before the
            # recurrence needs them.
            wemb_sb = const_pool.tile([128, 16, E], f16)         # 2 MB
            nc.sync.dma_start(
                wemb_sb[:], wembT.rearrange("(ko p) m -> p ko m", p=128)
            )
            wrec_sb = const_pool.tile([128, 8, G4], f16)         # 4 MB
            wrec_r = wrec.rearrange("(ko p) n -> p ko n", p=128)
            nc.gpsimd.dma_start(wrec_sb[:, 4:8, :], wrec_r[:, 4:8, :])
            nc.gpsimd.dma_start(wrec_sb[:, 0:4, :], wrec_r[:, 0:4, :])
            if has_bias:
                bias_sb = const_pool.tile([128, 512], f32)
                nc.sync.dma_start(bias_sb[:], biasg[:])

            c_sb = const_pool.tile([128, HP], f32)               # cell state

            def body():
                # h transposed state: hT[p, 32j+b] = h[b, 128j+p]
                hT = state_pool.tile([128, NJ * BL], f16, tag="hT")
                nc.vector.memzero(hT[:])
                nc.vector.memzero(c_sb[:])

                # ---- phase 1: xT = W_emb @ featsT (fp16), interleaved ----
                # Embedding chunk c (256 tokens = 8 steps of x input) is
                # emitted spread across the recurrence loop (8 MMs/step) so
                # the PE absorbs it during the per-step elementwise chain.
                NCH = 256
                NCHUNKS = 0 if mode == "rec" else (T * BL) // NCH
                featsT_r = featsT.rearrange("(ko p) n -> p ko n", p=128)
                emb_rhs = {}
                emb_ps = {}
                emb_copy_q = []

                def emb_dma(c, eng=None):
                    emb_rhs[c] = embin_pool.tile([128, 16, NCH], f16,
                                                 tag="embrhs",
                                                 name=f"embrhs_{c}")
                    (eng or nc.sync).dma_start(
                        emb_rhs[c][:],
                        featsT_r[:, :, c * NCH:(c + 1) * NCH],
                    )

                def emb_mms(c, m, ko_lo, ko_hi):
                    if (c, m) not in emb_ps:
                        emb_ps[(c, m)] = embps_pool.tile(
                            [128, 512], f32, tag="embps",
                            name=f"embps_{c}_{m}")[:, :NCH]
                    ps = emb_ps[(c, m)]
                    for ko in range(ko_lo, ko_hi):
                        nc.tensor.matmul(
                            ps[:],
                            wemb_sb[:, ko, m * 128:(m + 1) * 128],
                            emb_rhs[c][:, ko, :],
                            start=(ko == 0),
                            stop=(ko == 15),
                        )
                    if ko_hi == 16:
                        emb_copy_q.append((c, m))

                def emb_copies():
                    while emb_copy_q:
                        c, m = emb_copy_q.pop(0)
                        nc.vector.tensor_copy(
                            xT[:, m, c * NCH:(c + 1) * NCH],
                            emb_ps.pop((c, m))[:],
                        )

                if NCHUNKS and mode == "emb":
                    for c in range(NCHUNKS):
                        emb_dma(c)
                        for m in range(NJ):
                            emb_mms(c, m, 0, 16)
                            emb_copies()
                elif NCHUNKS:
                    # chunk 0 fully up-front (x(0..7) needs it immediately)
                    emb_dma(0)
                    emb_dma(1, eng=nc.gpsimd)
                    for m in range(NJ):
                        emb_mms(0, m, 0, 16)
                        emb_copies()

                def emb_quota(t):
                    """Per-step embedding emission inside the recurrence.
                    Chunk 1 spreads over steps 0-7; chunk c>=2 over steps
                    [8(c-2)+8 .. +15] (copies land just before x(8c) needs
                    them).  In-loop feats DMAs are emitted at t%8==1 so they
                    queue on the SP sequencer BEHIND step t's output DMA and
                    cannot jump ahead of the startup wrec transfers."""
                    if not NCHUNKS or mode == "emb":
                        return
                    if t % 8 == 0 and t // 8 + 2 < NCHUNKS:
                        emb_dma(t // 8 + 2)
                    if t < 4:
                        # chunk 1: one full m-tile per step (steps 0-3)
                        emb_mms(1, t, 0, 16)
                    else:
                        c = (t - 4) // 8 + 2
                        if c < NCHUNKS:
                            k = (t - 4) % 8
                            emb_mms(c, k // 2, 8 * (k % 2), 8 * (k % 2) + 8)

                # ---- phase 2: recurrence (fp16 col-tiled matmuls) ----
                # x-part MMs are emitted XAHEAD steps early so the PE has
                # independent work during step t's elementwise chain.
                XAHEAD = 1
                NT = 0 if mode == "emb" else T
                psgs = {}

                def emit_x(t):
                    fi = pgfi_pool.tile([128, 256], f32, tag="fi", name=f"psgfi_{t}")
                    go = pggo_pool.tile([128, 256], f32, tag="go", name=f"psggo_{t}")
                    psgs[t] = (fi, go)
                    for half, ps in ((0, fi), (1, go)):
                        for ko in range(4):
                            lhsT = xT[:, ko, t * BL:(t + 1) * BL]
                            for j in range(NJ):
                                nc.tensor.matmul(
                                    ps[32 * j:32 * (j + 1), :],
                                    lhsT,
                                    wrec_sb[:, ko + 4,
                                            j * 512 + 256 * half:
                                            j * 512 + 256 * half + 256],
                                    start=(ko == 0), stop=False,
                                    tile_position=(0, 32 * j),
                                    skip_group_check=True,
                                )

                for t in range(min(XAHEAD, NT)):
                    emit_x(t)
                for t in range(NT):
                    psg_fi, psg_go = psgs.pop(t)
                    # h-part MMs: (f,i) PSUM tile first so the sigmoid can
                    # start as soon as it closes; (g,o) tile runs while the
                    # sigmoid executes.
                    for half, ps in ((0, psg_fi), (1, psg_go)):
                        for ko in range(4):
                            lhsT = hT[:, 32 * ko:32 * (ko + 1)]
                            for j in range(NJ):
                                nc.tensor.matmul(
                                    ps[32 * j:32 * (j + 1), :],
                                    lhsT,
                                    wrec_sb[:, ko,
                                            j * 512 + 256 * half:
                                            j * 512 + 256 * half + 256],
                                    start=False, stop=(ko == 3),
                                    tile_position=(0, 32 * j),
                                    skip_group_check=True,
                                )
                    if t + XAHEAD < NT:
                        emit_x(t + XAHEAD)
                    emb_quota(t)
                    if has_bias:
                        nc.vector.tensor_add(psg_fi[:], psg_fi[:],
                                             bias_sb[:, 0:256])
                        nc.vector.tensor_add(psg_go[:], psg_go[:],
                                             bias_sb[:, 256:512])

                    hT_new = state_pool.tile([128, NJ * BL], f16, tag="hT")
                    if mode == "noew":
                        nc.vector.tensor_copy(hT_new[:], psg_fi[:, 0:HP])
                    else:
                        # gate order (f,i,g,o): f [0:128], i [128:256],
                        # g [256:384], o [384:512]
                        sfi = ew_pool.tile([128, 256], f32, tag="sfi")
                        nc.scalar.activation(
                            sfi[:], psg_fi[:],
                            mybir.ActivationFunctionType.Sigmoid,
                        )
                        gg = ew_pool.tile([128, HP], f32, tag="gg")
                        nc.scalar.activation(
                            gg[:], psg_go[:, 0:128],
                            mybir.ActivationFunctionType.Tanh,
                        )
                        sot = ew_pool.tile([128, HP], f16, tag="sot")
                        nc.scalar.activation(
                            sot[:], psg_go[:, 128:256],
                            mybir.ActivationFunctionType.Sigmoid,
                        )
                        fc = ew_pool.tile([128, HP], f32, tag="fc")
                        nc.vector.tensor_mul(fc[:], sfi[:, 0:128], c_sb[:])
                        ig = ew_pool.tile([128, HP], f32, tag="ig")
                        nc.vector.tensor_mul(ig[:], sfi[:, 128:256], gg[:])
                        nc.vector.tensor_add(c_sb[:], fc[:], ig[:])
                        tanhc = ew_pool.tile([128, HP], f16, tag="tanhc")
                        nc.scalar.activation(
                            tanhc[:], c_sb[:],
                            mybir.ActivationFunctionType.Tanh,
                        )
                        # transposes of s_o and tanh_c (full 128x128, f16)
                        tpo = pt_pool.tile([128, HP], f16, tag="tpo")
                        nc.tensor.matmul(
                            tpo[:], sot[:], ident16[:], is_transpose=True,
                        )
                        sotT = ew_pool.tile([128, HP], f16, tag="sotT")
                        nc.vector.tensor_copy(sotT[:], tpo[:])
                        tpc = pt_pool.tile([128, HP], f16, tag="tpc")
                        nc.tensor.matmul(
                            tpc[:], tanhc[:], ident16[:], is_transpose=True,
                        )
                        # hT = s_oT * tanh_cT  (f16 state, also the output)
                        nc.vector.tensor_mul(hT_new[:], sotT[:], tpc[:])
                    nc.sync.dma_start(out[t], hT_new[:])
                    emb_copies()
                    hT = hT_new

            if bench_loop:
                with tc.For_i(0, bench_loop, 1):
                    body()
            else:
                body()

    nc.compile()
    return nc


def _prep_inputs(feats_videos, W_embed, b_embed, W_ih, W_hh, b_ih, b_hh):
    """Host-side shard + relayout. Returns (in_maps, has_bias)."""
    f32 = np.float32
    # Combined recurrence weights: rows 0:511 = W_hh.T (h part),
    # rows 512:1023 = W_ih.T (x part).  Columns reordered to
    # col = jchunk*512 + gatepos*128 + h', gate order (f,i,g,o).
    W_cat = np.concatenate([W_hh.T, W_ih.T], axis=0).astype(f32)  # [1024, 2048]
    arr = W_cat.reshape(2 * E, 4, NJ, HP)       # [k, gate_orig, jchunk, h']
    arr = arr[:, [1, 0, 2, 3], :, :]            # gate order -> (f, i, g, o)
    wrec_np = np.ascontiguousarray(
        arr.transpose(0, 2, 1, 3).reshape(2 * E, G4)
    ).astype(np.float16)

    wembT_np = np.ascontiguousarray(W_embed.T).astype(np.float16)  # [F, E]

    # total gate bias, in the same [(j,b), (gatepos,h')] layout as psum
    b_g = (W_ih @ b_embed + b_ih + b_hh).astype(f32)        # [2048]
    has_bias = bool(np.any(b_g))
    bias_np = None
    if has_bias:
        bg = b_g.reshape(4, NJ, HP)[[1, 0, 2, 3], :, :]     # [gatepos, j, h']
        bias_tile = np.empty((128, 512), dtype=f32)
        for j in range(NJ):
            for gp in range(4):
                bias_tile[32 * j:32 * (j + 1), 128 * gp:128 * (gp + 1)] = bg[gp, j][None, :]
        bias_np = bias_tile

    in_maps = []
    for c in range(NC):
        fl = feats_videos[c * BL:(c + 1) * BL]              # [32, 64, 2048]
        featsT_np = np.ascontiguousarray(
            fl.transpose(2, 1, 0).reshape(F, T * BL)
        ).astype(np.float16)
        m = {"featsT": featsT_np, "wembT": wembT_np, "wrec": wrec_np}
        if has_bias:
            m["biasg"] = bias_np
        in_maps.append(m)
    return in_maps, has_bias


def _unshard(results):
    """results[c]["out"] is [T, 128, 128] f16 with out[t, p, 32j+b] =
    h[b, 128j+p].  Returns [B, T, H] f32."""
    outs = []
    for c in range(NC):
        r = np.asarray(results[c]["out"])
        o = (r.reshape(T, HP, NJ, BL)
             .transpose(3, 0, 2, 1)
             .reshape(BL, T, H))
        outs.append(o)
    return np.concatenate(outs, axis=0).astype(np.float32)


def kernel(feats_videos, W_embed, b_embed, W_ih, W_hh, b_ih, b_hh):
    import sys
    if "/opt/trn_rl_repo" not in sys.path:
        sys.path.insert(0, "/opt/trn_rl_repo")
    from concourse.bass_utils import run_bass_kernel_spmd

    feats_videos = np.asarray(feats_videos, dtype=np.float32)
    W_embed = np.asarray(W_embed, dtype=np.float32)
    b_embed = np.asarray(b_embed, dtype=np.float32)
    W_ih = np.asarray(W_ih, dtype=np.float32)
    W_hh = np.asarray(W_hh, dtype=np.float32)
    b_ih = np.asarray(b_ih, dtype=np.float32)
    b_hh = np.asarray(b_hh, dtype=np.float32)

    in_maps, has_bias = _prep_inputs(
        feats_videos, W_embed, b_embed, W_ih, W_hh, b_ih, b_hh
    )
    if has_bias not in _prog_cache:
        _prog_cache[has_bias] = _build_program(has_bias)
    nc = _prog_cache[has_bias]

    res = run_bass_kernel_spmd(nc, in_maps, list(range(NC)))
    return _unshard(res.results)
